# revision 80
# baseline (speedup 1.0000x reference)
"""Trainium2 Bass kernel for nn_Attention_48601849922045.

GQA attention layer (B=2, S=2048, D=2048, H=32 q-heads, KV=8 kv-heads, HD=64)
with llama RoPE, causal softmax, and output projection.

Sharding: tensor-parallel over heads across 8 cores - each core owns one KV
group (1 kv head + its 4 q heads).  x is replicated; per core:

  1. QKV projection, weights-stationary, two passes over the resident x
     chunks (e-tiles {q0,q1} then {k|v}) so it needs only 2 PSUM banks.
  2. RoPE applied in [e, n] layout: partition pair-swap via two strided
     SBUF->SBUF DMAs, then 3 large DVE ops against host-built cos/sin
     tables.  v (no rope) is PE-transposed to natural [t, hd] layout and
     augmented with a ones column so the PV matmul also produces the
     softmax denominator.
  3. Attention per (b, head-PAIR): the local q heads are processed two at
     a time using PE row tiling - kr keeps k duplicated on partitions
     0-63 and 64-127, and qr keeps the pair's heads on the two halves, so
     the even head's scores matmul runs on row tile (0,0) while the odd
     head's runs concurrently on row tile (64,0).  This halves the scores
     phase (K=64 would otherwise idle half the PE array).  Both heads'
     score strips land in one [P, 2, JW] PSUM tile (double-buffered),
     giving a single exp per strip; the exp is sliced to start at the
     strip's causal column, skipping the fully-masked region.  PV is
     K=128 (full array), software-pipelined TWO strips behind scores,
     with the filler pull placed between scores(i) and PV(i-2) so the
     PE never sits at a PV whose exp hasn't finished.
  4. Eight small per-(b,h) bf16 AllToAlls reshard o from head-sharded to
     row-sharded.  The reshard staging tile orT is SPLIT per head pair:
     a single tile would make every outproj matmul wait on the LAST
     collective (dependencies are tracked per tile), killing the tail.
     Each pair's last-j drains are DEFERRED into the next pair's first
     j-block (urgent filler) so the next pair's scores are not queued
     behind the drain chain's ones-matmuls; the pair's collectives are
     emitted right after those drains.
  5. Row-parallel output projection with the full wo resident in SBUF
     (loaded once), micro-interleaved into the attention phases via
     filler generators (~1us PE quanta) with start_delay tuned so the
     first filler matmul never waits on an unlanded collective; the
     b=1 waves run hp->s->mt->db (4 matmuls share each orT stationary)
     with all 8 PSUM banks as accumulators, hp0 waves before the last
     collective, hp1 after.

Scheduling notes (hard-won): every engine queue executes in order, so a
filler matmul whose inputs aren't ready stalls every attention matmul
behind it (hence start_delay and the orT split); cross-engine deps
resolve via the producing engine's queue tick, so PE work emitted after
a collective waits for GpSimd to pass that collective's staging even if
its data landed long ago (hence the waves/db3 emission BEFORE
a2a_pair(1,1)); the exp chain on ACT is the attention pipeline clock,
so nothing else may sit on the ACT queue mid-kernel; both collectives
of a pair are issued before either's orT staging DMAs so the second
doesn't queue behind the first's wait; the first qkv block's wT chunk
loads are interleaved with its xg chunks and the rope tables load after
them, so the DMA rings deliver the ramp's operands first.

Host side only shards/transposes inputs and concatenates the 8 output
row-shards.
"""

import os

import numpy as np
import ml_dtypes

import concourse.bass as bass
import concourse.bacc as bacc
import concourse.tile as tile
import concourse.mybir as mybir
from concourse.bass_utils import run_bass_kernel_spmd

P = 128
B, S, D = 2, 2048, 2048
H, KV, HD = 32, 8, 64
NCORES = 8
HL = H // NCORES          # 4 local q heads
BS = B * S                # 4096 rows
EQ, EK, EV = HL * HD, HD, HD
E3 = EQ + EK + EV         # 384 = 3 PE tiles of 128
ET = E3 // P              # 3 e-tiles (0,1: q heads, 2: k|v stacked)
CH = D // P               # 16 contraction chunks
STC = S // P              # 16 t-chunks per batch
NBW = 512                 # qkv n-block width
NBB = S // NBW            # 4 n-blocks per batch
JW = 512                  # attention n-block width
JB = S // JW
RSH = BS // NCORES // B   # 256 rows per (core, batch)
VAW = P                   # v-aug stride: ones at col 64, zero-padded
                          # to 128 cols so the PV stationary gets FWL

F32 = mybir.dt.float32
BF16 = mybir.dt.bfloat16

_CACHE = {}


def _build_nc():
    nc = bacc.Bacc("TRN2", target_bir_lowering=False, debug=False,
                   num_devices=NCORES)

    # xT and woT arrive pre-tiled so every SBUF tile is one contiguous
    # 128KB DRAM block (1KB-row strided loads only reach ~50GB/s)
    xT = nc.dram_tensor("xT", [CH, B * NBB, P, NBW], BF16,
                        kind="ExternalInput").ap()
    wT = nc.dram_tensor("wT", [D, E3], BF16, kind="ExternalInput").ap()
    woT = nc.dram_tensor("woT", [CH, D // JW, P, JW], BF16,
                         kind="ExternalInput").ap()
    cosT = nc.dram_tensor("cosT", [P, S], BF16, kind="ExternalInput").ap()
    sinPM = nc.dram_tensor("sinPM", [P, S], BF16, kind="ExternalInput").ap()
    mask2 = nc.dram_tensor("mask2", [P, 2 * P], BF16,
                           kind="ExternalInput").ap()
    ident = nc.dram_tensor("ident", [P, P], BF16, kind="ExternalInput").ap()
    out = nc.dram_tensor("out", [B * RSH, D], F32, kind="ExternalOutput").ap()

    with tile.TileContext(nc) as tc:
        with (
            tc.tile_pool(name="const", bufs=1) as const,
            tc.tile_pool(name="dram", bufs=1, space="DRAM") as dram,
            # PSUM plan (8 banks): 2x 1-bank accumulators (qkv passes,
            # vtp, outproj), two 2-bank scores slots (both heads of the
            # pair, double-buffered), two 1-bank o accumulators
            tc.tile_pool(name="psacc", bufs=2, space="PSUM") as psacc,
            tc.tile_pool(name="pssp", bufs=2, space="PSUM") as pssp,
            tc.tile_pool(name="pso", bufs=1, space="PSUM") as pso,
            tc.tile_pool(name="xg", bufs=CH) as xgp,
            tc.tile_pool(name="drain", bufs=3) as drainp,
            tc.tile_pool(name="ptp", bufs=4) as ptp,
            tc.tile_pool(name="nrm", bufs=2) as nrm,
            tc.tile_pool(name="otp", bufs=2) as otp,
            tc.tile_pool(name="wos", bufs=4 * CH) as wosp,
            tc.tile_pool(name="orp", bufs=1) as orp,
            tc.tile_pool(name="outs", bufs=2) as outsp,
        ):
            # ---- constants resident in SBUF ----
            # wT chunk loads are interleaved with the first qkv block's xg
            # chunks (emitted inside the stepped generator below) and the
            # tables load after that block's matmul phase, so the DMA rings
            # deliver the first block's operands first (ramp, not 14us)
            wT_sb = [const.tile([P, E3], BF16, name=f"wT{c}")
                     for c in range(CH)]
            cos_sb = const.tile([P, S], BF16)
            sin_sb = const.tile([P, S], BF16)
            mask2_sb = const.tile([P, 2, P], BF16)
            id_sb = const.tile([P, P], BF16)
            ones_sb = const.tile([1, HD], BF16)
            nc.vector.memset(ones_sb, 1.0)

            sw_sh = const.tile([P, S], BF16, name="sw_sh")
            st = {}
            for b in range(B):
                st[b] = {
                    # raw projections, [e, n] layout, 2 q-head pairs + k|v
                    "q2": [const.tile([P, S], BF16, name=f"q2_{b}{i}")
                           for i in range(2)],
                    "kv": const.tile([P, S], BF16, name=f"kv{b}"),
                    # post-rope
                    "qr": [const.tile([P, S], BF16, name=f"qr{b}{i}")
                           for i in range(2)],
                    # k stored twice (partitions 0-63 and 64-127) so the
                    # pair's two heads can run on both PE row tiles
                    "kr": const.tile([P, S], BF16, name=f"kr{b}"),
                    "sw": sw_sh,
                    "vA": const.tile([P, STC * VAW], BF16, name=f"vA{b}"),
                }
                nc.vector.memset(st[b]["vA"], 0.0)
                ones_col = st[b]["vA"].rearrange(
                    "p (t w) -> p t w", w=VAW)[:, :, HD:HD + 1]
                nc.vector.memset(ones_col, 1.0)

            a2a_in = dram.tile([B, HL, NCORES, HD, RSH], BF16)
            a2a_out = dram.tile([B, HL, NCORES, HD, RSH], BF16)

            # orT split per (batch, head pair, SOURCE CORE): dependency
            # tracking is tile-granular, so a coarser tile would chain
            # every outproj matmul to the LAST of the pair's 16 staging
            # DMAs; per-s tiles let the s=0 matmuls start after the
            # first staging lands
            orT = {(b, hp, s): orp.tile([P, RSH], BF16,
                                        name=f"orT{b}{hp}{s}",
                                        tag=f"orT{b}{hp}{s}")
                   for b in range(B) for hp in range(2)
                   for s in range(NCORES)}

            def a2a_pair(b, hp):
                """Issue both collectives of a pair, then both orT
                stagings - so the second collective isn't queued behind
                the first's completion wait.  Stagings are s-major,
                head-minor so each s-tile completes as early as
                possible."""
                for h in (2 * hp, 2 * hp + 1):
                    nc.gpsimd.collective_compute(
                        "AllToAll",
                        mybir.AluOpType.bypass,
                        replica_groups=[list(range(NCORES))],
                        ins=[a2a_in[b, h].opt()],
                        outs=[a2a_out[b, h].opt()],
                    )
                for s in range(NCORES):
                    for h in (2 * hp, 2 * hp + 1):
                        k = h % 2
                        nc.gpsimd.dma_start(
                            out=orT[b, hp, s][k * HD:(k + 1) * HD, :],
                            in_=a2a_out[b, h, s, :, :])

            def run(gen):
                for _ in gen:
                    pass

            class Filler:
                def __init__(self, gens, start_delay=0):
                    self.gens = list(gens)
                    self.idx = 0
                    self.delay = start_delay

                def __call__(self, n=1):
                    if self.delay > 0:
                        self.delay -= 1
                        return
                    emitted = 0
                    while emitted < n and self.idx < len(self.gens):
                        try:
                            next(self.gens[self.idx])
                            emitted += 1
                        except StopIteration:
                            self.idx += 1

                def drain(self, upto):
                    """Fully emit generators 0..upto."""
                    while self.idx <= upto and self.idx < len(self.gens):
                        try:
                            next(self.gens[self.idx])
                        except StopIteration:
                            self.idx += 1

            class Chain:
                """Pull `urgent` generators at a high rate until they
                run dry, then delegate to the `rest` filler."""

                def __init__(self, urgent, rest, rate=3):
                    self.urgent = list(urgent)
                    self.idx = 0
                    self.rest = rest
                    self.rate = rate

                def __call__(self, n=1):
                    if self.idx < len(self.urgent):
                        emitted = 0
                        while (emitted < self.rate
                               and self.idx < len(self.urgent)):
                            try:
                                next(self.urgent[self.idx])
                                emitted += 1
                            except StopIteration:
                                self.idx += 1
                        return
                    if self.rest is not None:
                        self.rest(n)

            # ---- emission order = engine-queue order = priority ----
            # qkv(0,0) runs plain; later blocks fill attention PE gaps.
            # global n-blocks 1..6 are PAIRED (wT is batch-independent):
            # each pass keeps one wT slice stationary for two n-blocks'
            # matmuls, halving the exposed LDWEIGHTS cost.
            qargs = (nc, xT, wT_sb, cos_sb, sin_sb, id_sb, st, xgp, psacc,
                     drainp)
            qgens = Filler(
                [_qkv_block(*qargs, [nbg]) for nbg in range(B * NBB)])
            # ramp: step block (0,0)'s chunk loop manually, issuing wT[c]
            # just ahead of chunk c's matmuls; tables go after the chunk
            # loop (rope/mask/vtp consume them later)
            g0 = qgens.gens[0]
            for c in range(CH):
                nc.gpsimd.dma_start(out=wT_sb[c],
                                    in_=wT[c * P:(c + 1) * P, :])
                next(g0)
            nc.scalar.dma_start(out=cos_sb, in_=cosT)
            nc.scalar.dma_start(out=sin_sb, in_=sinPM)
            nc.scalar.dma_start(out=mask2_sb, in_=mask2)
            nc.scalar.dma_start(out=id_sb, in_=ident)
            # every j-block's drains ride as urgent pulls in the NEXT
            # j-block's strips (threaded across calls via incoming), and
            # each pair's collective is emitted right after its last
            # drains complete - so no j's scores ever queue behind the
            # drain chain's ones-matmuls
            qgens.drain(0)  # rest of qkv nbg0
            dr = None
            for nb in range(NBB):
                dr = _attn_pair(nc, 0, 0, st[0], mask2_sb, ones_sb,
                                a2a_in, pssp, pso, ptp, nrm, otp,
                                j_range=[nb], filler=qgens,
                                defer_drains=True, incoming=dr)
                if nb + 1 < NBB:
                    qgens.drain(nb + 1)  # qkv(0,nb+1) before j=nb+1
            dr = _attn_pair(nc, 0, 1, st[0], mask2_sb, ones_sb, a2a_in,
                            pssp, pso, ptp, nrm, otp, j_range=[0],
                            filler=qgens, defer_drains=True, incoming=dr)
            a2a_pair(0, 0)
            dr01 = _attn_pair(nc, 0, 1, st[0], mask2_sb, ones_sb, a2a_in,
                              pssp, pso, ptp, nrm, otp, j_range=[1, 2, 3],
                              filler=qgens, defer_drains=True, incoming=dr)
            qgens.drain(7)  # rest of batch-1 qkv
            # the full wo loads once, on GpSimd, emitted BEFORE the next
            # collectives so no orT wait sits ahead of them
            wosr = {db: _load_wos(nc, woT, wosp, db, nc.gpsimd)
                    for db in range(4)}
            # batch 1 attention; fill with batch-0 output projection.
            # start_delay covers orT[0,1]'s dependency on batch 0's last
            # AllToAll so the first filler matmul doesn't stall the PE
            # queue mid-pair.
            f_op0 = Filler(
                [_outproj_gen(nc, db, 0, out, orT, wosr[db], psacc, outsp)
                 for db in (0, 1)],
                start_delay=23)
            dr = _attn_pair(nc, 1, 0, st[1], mask2_sb, ones_sb, a2a_in,
                            pssp, pso, ptp, nrm, otp, j_range=[0],
                            filler=f_op0, defer_drains=True, incoming=dr01)
            a2a_pair(0, 1)
            dr10 = _attn_pair(nc, 1, 0, st[1], mask2_sb, ones_sb, a2a_in,
                              pssp, pso, ptp, nrm, otp, j_range=[1, 2, 3],
                              filler=f_op0, defer_drains=True, incoming=dr)
            f_op0(10000)
            f_op1 = Filler(
                [_outproj_gen(nc, db, 0, out, orT, wosr[db], psacc, outsp)
                 for db in (2,)],
                start_delay=27)
            dr = _attn_pair(nc, 1, 1, st[1], mask2_sb, ones_sb, a2a_in,
                            pssp, pso, ptp, nrm, otp, j_range=[0],
                            filler=f_op1, defer_drains=True, incoming=dr10)
            a2a_pair(1, 0)
            _attn_pair(nc, 1, 1, st[1], mask2_sb, ones_sb, a2a_in,
                       pssp, pso, ptp, nrm, otp, j_range=[1, 2, 3],
                       filler=f_op1, incoming=dr)
            f_op1(10000)
            # batch-0 db3 pass: collective-independent PE work that runs
            # during the final AllToAlls' flight
            run(_outproj_gen(nc, 3, 0, out, orT, wosr[3], psacc, outsp))
            # batch-1 outproj: all 8 PSUM banks are free now - allocate
            # every (db, mt) accumulator explicitly so no wave serializes
            # behind another's drain.  hp -> s -> mt -> db order: 4
            # consecutive matmuls share the orT stationary slice.
            # CRITICAL: the hp0 waves are emitted BEFORE a2a_pair(1, 1) -
            # cross-engine deps resolve via the producing engine's queue
            # tick, so anything emitted after the collective would wait
            # for GpSimd to pass the last staging even though orT[1,0]
            # landed long ago (30us PE idle otherwise).
            spbA = pssp.tile([P, 2, JW], F32, name="sp", tag="sp")
            spbB = pssp.tile([P, 2, JW], F32, name="sp", tag="sp")
            opb = {
                0: [pso.tile([P, JW], F32, name="opA", tag="oe"),
                    pso.tile([P, JW], F32, name="opB", tag="oo")],
                1: [psacc.tile([P, JW], F32, name="opC", tag="ps"),
                    psacc.tile([P, JW], F32, name="opD", tag="ps")],
                2: [spbA[:, 0, :], spbA[:, 1, :]],
                3: [spbB[:, 0, :], spbB[:, 1, :]],
            }
            MT = RSH // P

            def waves(hp):
                for si, s_ in enumerate(range(NCORES)):
                    for mt in range(MT):
                        lhsT = orT[1, hp, s_][:, mt * P:(mt + 1) * P]
                        for db in range(4):
                            nc.tensor.matmul(
                                opb[db][mt][:, :],
                                lhsT=lhsT,
                                rhs=wosr[db][2 * s_ + hp],
                                start=(hp == 0 and si == 0),
                                stop=(hp == 1 and si == NCORES - 1))

            waves(0)
            a2a_pair(1, 1)
            waves(1)
            for db in range(4):
                for mt in range(MT):
                    osb = outsp.tile([P, JW], F32, name="osb")
                    nc.vector.tensor_copy(out=osb, in_=opb[db][mt])
                    eng = (nc.sync, nc.scalar)[(2 * db + mt) % 2]
                    eng.dma_start(
                        out=out[RSH + mt * P:RSH + (mt + 1) * P,
                                db * JW:(db + 1) * JW],
                        in_=osb)

    nc.compile()
    return nc


def _qkv_block(nc, xT, wT_sb, cos_sb, sin_sb, id_sb, st, xgp, psacc,
               drainp, nbgs):
    """Weights-stationary projection for one or two 512-column global
    n-blocks, followed by their rope, k-duplicate, and v-transpose.

    Single block: one pass over the e-tiles {q0,q1} then {k|v} (2 PSUM
    banks, original ramp-friendly structure: one yield per chunk).
    Pair: one pass PER e-tile with the wT slice kept stationary across
    both blocks' matmuls (halves the exposed LDWEIGHTS cost; still 2
    PSUM banks, one accumulator per block)."""
    TPB = NBW // P  # t-chunks per n-block
    blks = [(nbg // NBB, nbg % NBB, nbg) for nbg in nbgs]
    xgs = {nbg: [] for nbg in nbgs}

    def load_xg(nbg, c, k):
        xg = xgp.tile([P, NBW], BF16)
        xgs[nbg].append(xg)
        b = nbg // NBB
        if nbg == 0:
            eng = (nc.sync, nc.gpsimd, nc.scalar)[c % 3]
        elif (b == 0 and c % 2) or (len(nbgs) > 1 and (c + k) % 2):
            eng = nc.gpsimd
        else:
            eng = nc.sync
        eng.dma_start(out=xg, in_=xT[c, nbg])
        return xg

    if len(nbgs) == 1:
        b, nb, nbg = blks[0]
        stb = st[b]
        n0 = nb * NBW
        sw = stb["sw"]
        psA = [psacc.tile([P, NBW], F32, name=f"psA{e}", tag="ps")
               for e in range(2)]
        for c in range(CH):
            xg = load_xg(nbg, c, 0)
            for e in range(2):
                nc.tensor.matmul(
                    psA[e][:, :],
                    lhsT=wT_sb[c][:, e * P:(e + 1) * P],
                    rhs=xg,
                    start=(c == 0), stop=(c == CH - 1))
            yield
        for e in range(2):
            nc.vector.tensor_copy(out=stb["q2"][e][:, n0:n0 + NBW],
                                  in_=psA[e])
        yield
        psB = psacc.tile([P, NBW], F32, name="psB", tag="ps")
        for c in range(CH):
            nc.tensor.matmul(
                psB[:, :],
                lhsT=wT_sb[c][:, 2 * P:3 * P],
                rhs=xgs[nbg][c],
                start=(c == 0), stop=(c == CH - 1))
            if c % 2:
                yield
        nc.vector.tensor_copy(out=stb["kv"][:, n0:n0 + NBW], in_=psB)
        yield
    else:
        for e in range(3):
            ps = [psacc.tile([P, NBW], F32, name=f"ps{e}{k}", tag="ps")
                  for k in range(2)]
            for c in range(CH):
                for k, (b, nb, nbg) in enumerate(blks):
                    xg = (load_xg(nbg, c, k) if e == 0 else xgs[nbg][c])
                    nc.tensor.matmul(
                        ps[k][:, :],
                        lhsT=wT_sb[c][:, e * P:(e + 1) * P],
                        rhs=xg,
                        start=(c == 0), stop=(c == CH - 1))
                yield
            for k, (b, nb, nbg) in enumerate(blks):
                n0 = nb * NBW
                dst = (st[b]["q2"][e] if e < 2 else st[b]["kv"])
                nc.vector.tensor_copy(out=dst[:, n0:n0 + NBW], in_=ps[k])
            yield

    # rope + k-dup + v-transpose per block
    for b, nb, nbg in blks:
        stb = st[b]
        n0 = nb * NBW
        sw = stb["sw"]
        for e in range(2):
            _rope_t(nc, drainp, stb["q2"][e], stb["qr"][e], sw, cos_sb,
                    sin_sb, P, n0)
            yield
        _rope_t(nc, drainp, stb["kv"], stb["kr"], sw, cos_sb, sin_sb,
                HD, n0)
        nc.sync.dma_start(out=stb["kr"][HD:P, n0:n0 + NBW],
                          in_=stb["kr"][0:HD, n0:n0 + NBW])
        yield

        # v: PE transpose to natural [t, hd] + ones column
        vAv = stb["vA"].rearrange("p (t w) -> p t w", w=VAW)[:, :, 0:HD]
        vtp = psacc.tile([P, TPB * HD], BF16, name="vtp", tag="ps",
                         padded_shape=[P, 2 * TPB * HD])
        for tl in range(TPB):
            t = nb * TPB + tl
            nc.tensor.transpose(vtp[:, tl * HD:(tl + 1) * HD],
                                stb["kv"][HD:P, t * P:(t + 1) * P],
                                id_sb[HD:P, HD:P])
        nc.vector.tensor_copy(
            out=vAv[:, nb * TPB:(nb + 1) * TPB, :],
            in_=vtp.rearrange("p (t w) -> p t w", w=HD))
        yield


def _rope_t(nc, drainp, src, dst, sw, cos_sb, sin_sb, rows, n0):
    """dst[0:rows, n0:n0+NBW] = rope(src[...]) in [e, n] layout.

    Pairs are adjacent partitions; sw is scratch for the pair-swapped copy.
    cos_sb[p, s] = cos(ang[s, p//2 % 32]); sin_sb has the -/+ sign baked in:
    sin_sb[2i] = -sin, sin_sb[2i+1] = +sin."""
    n1 = n0 + NBW
    # sw[2i] = src[2i+1], sw[2i+1] = src[2i]
    nc.sync.dma_start(out=sw[0:rows:2, n0:n1], in_=src[1:rows:2, n0:n1])
    nc.sync.dma_start(out=sw[1:rows:2, n0:n1], in_=src[0:rows:2, n0:n1])
    t1 = drainp.tile([P, NBW], BF16, name="t1", tag="t1", bufs=1)
    t2 = drainp.tile([P, NBW], BF16, name="t2", tag="t2", bufs=1)
    nc.vector.tensor_mul(t1[0:rows], src[0:rows, n0:n1],
                         cos_sb[0:rows, n0:n1])
    nc.vector.tensor_mul(t2[0:rows], sw[0:rows, n0:n1],
                         sin_sb[0:rows, n0:n1])
    nc.vector.tensor_add(dst[0:rows, n0:n1], t1[0:rows], t2[0:rows])


def _attn_pair(nc, b, hp, stb, mask2_sb, ones_sb, a2a_in, pssp, pso, ptp,
               nrm, otp, j_range=None, filler=None, fill_every=1,
               defer_drains=False, incoming=None):
    """Causal attention for one (batch, head pair).  The pair's two heads
    run concurrently on the PE's two 64-row tiles during scores; one
    [P, 2, JW] PSUM tile holds both heads' strip so a single exp drains it.
    PV is emitted one strip behind scores (software pipelining) and sp is
    double-buffered, so the PE queue never sits directly behind the exp.
    filler() emits ~1us of foreign PE work per strip to cover the
    exp-bound slack."""
    qr, kr, vA = stb["qr"], stb["kr"], stb["vA"]
    qp = qr[hp]
    carry = list(incoming) if incoming else []
    for j in (range(JB) if j_range is None else j_range):
        n0 = j * JW
        ni = (n0 + JW) // P
        o_e = pso.tile([P, JW], F32, name="o_e", tag="oe")
        o_o = pso.tile([P, JW], F32, name="o_o", tag="oo")
        pend = []
        for i in range(ni):
            d = max(0, i * P - n0)
            sp = pssp.tile([P, 2, JW], F32, name="sp", tag="sp")
            # even head on row tile (0,0), odd head on (64,0) - concurrent
            nc.tensor.matmul(
                sp[:, 0, d:JW],
                lhsT=kr[0:HD, i * P:(i + 1) * P],
                rhs=qp[0:HD, n0 + d:n0 + JW],
                start=True, stop=True)
            nc.tensor.matmul(
                sp[:, 1, d:JW],
                lhsT=kr[HD:P, i * P:(i + 1) * P],
                rhs=qp[HD:P, n0 + d:n0 + JW],
                start=True, stop=True)
            pt = ptp.tile([P, 2, JW], BF16, name="pt")
            # exp starts at the strip's causal column: cols < d are fully
            # masked and never read downstream
            nc.scalar.activation(out=pt[:, :, d:JW], in_=sp[:, :, d:JW],
                                 func=mybir.ActivationFunctionType.Exp)
            if i * P >= n0:
                # only the [128,128] strip at cols [d, d+128) is partial;
                # both heads' slots share one masked multiply
                nc.vector.tensor_mul(
                    pt[:, :, d:d + P], pt[:, :, d:d + P], mask2_sb)
            # filler sits BETWEEN this strip's scores and an older strip's
            # PV in the PE queue; PV trails TWO strips behind scores so
            # its exp has ~2 strip-times to finish before the PE reaches
            # the PV head-of-line
            if filler is not None and i % fill_every == 0:
                filler()
            # urgent: the previous j-block's (or pair's) deferred drain
            # steps interleave here so their ones-matmuls never park at
            # the head of the PE queue ahead of this j's scores
            pulled = 0
            while carry and pulled < 3:
                try:
                    next(carry[0])
                    pulled += 1
                except StopIteration:
                    carry.pop(0)
            pend.append((i, pt))
            if len(pend) > 2:
                _pv(nc, vA, o_e, o_o, pend.pop(0), n0, ni)
        for prev in pend:
            _pv(nc, vA, o_e, o_o, prev, n0, ni)
        for g in carry:          # safety flush (normally already empty)
            for _ in g:
                pass
        carry = []
        gens = [_o_drain(nc, b, 2 * hp, o_e, ones_sb, a2a_in, nrm, otp, n0),
                _o_drain(nc, b, 2 * hp + 1, o_o, ones_sb, a2a_in, nrm, otp,
                         n0)]
        if j == (JB - 1 if j_range is None else j_range[-1]):
            if defer_drains:
                # the caller threads these into the NEXT call's strips
                return gens
            for g in gens:
                for _ in g:
                    pass
        else:
            carry = gens


def _pv(nc, vA, o_e, o_o, prev, n0, ni):
    i, pt = prev
    d = max(0, i * P - n0)
    nc.tensor.matmul(
        o_e[:, d:JW],
        lhsT=vA[:, i * VAW:(i + 1) * VAW],
        rhs=pt[:, 0, d:JW],
        start=(i == 0), stop=(i == ni - 1))
    nc.tensor.matmul(
        o_o[:, d:JW],
        lhsT=vA[:, i * VAW:(i + 1) * VAW],
        rhs=pt[:, 1, d:JW],
        start=(i == 0), stop=(i == ni - 1))


def _o_drain(nc, b, h, o_ps, ones_sb, a2a_in, nrm, otp, n0):
    """Normalize one head's o for this n-block and stage it for the
    AllToAll (generator, ~4 quanta).  Avoids GpSimd (its queue must stay
    free to block on collective waits) and ACT (the exp chain): 1/l on
    DVE, then a K=1 ones-matmul broadcasts r into the unused rows
    64..127 of the o bank."""
    l_sb = nrm.tile([1, JW], F32, name="l_sb", tag="l")
    nc.vector.tensor_copy(out=l_sb, in_=o_ps[HD:HD + 1, :])
    r = nrm.tile([1, JW], F32, name="r", tag="r")
    nc.vector.reciprocal_approx_fast(out=r, in_=l_sb)
    yield
    rb16 = nrm.tile([1, JW], BF16, name="rb16", tag="r16")
    nc.vector.tensor_copy(out=rb16, in_=r)
    nc.tensor.matmul(o_ps[HD:HD + HD, :], lhsT=ones_sb, rhs=rb16,
                     start=True, stop=True)
    yield
    # DVE reads at most one PSUM operand: stage o in SBUF first
    o_f = otp.tile([HD, JW], F32, name="o_f", tag="o_f", bufs=1)
    nc.vector.tensor_copy(out=o_f, in_=o_ps[0:HD, :])
    ot = otp.tile([HD, JW], BF16, name="ot")
    nc.vector.tensor_mul(ot, o_f, o_ps[HD:HD + HD, :])
    yield
    for half in range(JW // RSH):
        dest = (n0 + half * RSH) // RSH
        nc.sync.dma_start(
            out=a2a_in[b, h, dest, :, :],
            in_=ot[:, half * RSH:(half + 1) * RSH])
    yield


def _load_wos(nc, woT, wosp, db, eng):
    """Stage one column block's wo chunks; returns the 16 tiles."""
    wos = {}
    for c in range(CH):
        w = wosp.tile([P, JW], BF16, name=f"wos{db}_{c}", tag="wos")
        eng.dma_start(out=w, in_=woT[c, db])
        wos[c] = w
    return wos


def _outproj_wave(nc, db, b, hp, out, orT, wos, ops, outsp):
    """One head-pair wave of a (column-block, batch) o @ wo.T pass into the
    caller-provided pair of PSUM accumulators."""
    MT = RSH // P  # 2 row tiles per batch
    for si, s in enumerate(range(NCORES)):
        c = 2 * s + hp
        for mt in range(MT):
            nc.tensor.matmul(
                ops[mt][:, :],
                lhsT=orT[b, hp][:, s * RSH + mt * P:s * RSH + (mt + 1) * P],
                rhs=wos[c],
                start=(hp == 0 and si == 0),
                stop=(hp == 1 and si == NCORES - 1))
    if hp == 1:
        for mt in range(MT):
            osb = outsp.tile([P, JW], F32, name="osb")
            nc.vector.tensor_copy(out=osb, in_=ops[mt])
            eng = (nc.sync, nc.scalar)[(2 * db + mt) % 2]
            eng.dma_start(
                out=out[b * RSH + mt * P:b * RSH + (mt + 1) * P,
                        db * JW:(db + 1) * JW],
                in_=osb)


def _outproj_gen(nc, db, b, out, orT, wos, psacc, outsp):
    """Full (column-block, batch) pass as a generator, hp-major: all the
    pair-0 contributions (whose collective landed long ago) come first,
    so early filler pulls never sit on the pair-1 collective's semaphore.
    Holds both mt accumulators (2 PSUM banks) for the pass."""
    MT = RSH // P
    ops = [psacc.tile([P, JW], F32, name=f"op{mt}", tag="ps")
           for mt in range(MT)]
    for hp in range(2):
        for mt in range(MT):
            for si, s in enumerate(range(NCORES)):
                c = 2 * s + hp
                nc.tensor.matmul(
                    ops[mt][:, :],
                    lhsT=orT[b, hp, s][:, mt * P:(mt + 1) * P],
                    rhs=wos[c],
                    start=(hp == 0 and si == 0),
                    stop=(hp == 1 and si == NCORES - 1))
                if si % 2:
                    yield
    for mt in range(MT):
        osb = outsp.tile([P, JW], F32, name="osb")
        nc.vector.tensor_copy(out=osb, in_=ops[mt])
        nc.sync.dma_start(
            out=out[b * RSH + mt * P:b * RSH + (mt + 1) * P,
                    db * JW:(db + 1) * JW],
            in_=osb)
        yield


def _host_prep(x, freqs_cis, wq, wk, wv, wo):
    """Build per-core input maps (numpy only)."""
    x = np.asarray(x, np.float32)
    freqs_cis = np.asarray(freqs_cis, np.float32)
    wq = np.asarray(wq, np.float32)
    wk = np.asarray(wk, np.float32)
    wv = np.asarray(wv, np.float32)
    wo = np.asarray(wo, np.float32)
    bf = ml_dtypes.bfloat16

    # pre-tiled: [c, nb, p, n] with each (c, nb) block contiguous
    xT = np.ascontiguousarray(
        x.reshape(BS, D).T.reshape(CH, P, B * NBB, NBW)
        .transpose(0, 2, 1, 3)).astype(bf)
    woT = np.ascontiguousarray(
        wo.T.reshape(CH, P, D // JW, JW).transpose(0, 2, 1, 3)).astype(bf)
    scale = 1.0 / np.sqrt(np.float32(HD))

    # transposed-layout rope tables: [p, s]
    cos = freqs_cis[:, :, 0]   # [S, 32]
    sin = freqs_cis[:, :, 1]
    pair = (np.arange(P) // 2) % (HD // 2)
    sign = np.where(np.arange(P) % 2 == 0, -1.0, 1.0).astype(np.float32)
    cosT = np.ascontiguousarray(cos[:, pair].T).astype(bf)    # [P, S]
    sinPM = (np.ascontiguousarray(sin[:, pair].T) * sign[:, None]).astype(bf)

    # upper triangle incl diagonal: valid where col >= row; duplicated so
    # one DVE op masks both heads' slots of a [P, 2, P] strip
    maskb = (np.arange(P)[None, :] >= np.arange(P)[:, None]).astype(bf)
    mask2 = np.ascontiguousarray(
        np.stack([maskb, maskb], axis=1).reshape(P, 2 * P))

    identm = np.eye(P, dtype=bf)

    in_maps = []
    for r in range(NCORES):
        wq_r = wq[r * EQ:(r + 1) * EQ] * scale
        wk_r = wk[r * EK:(r + 1) * EK]
        wv_r = wv[r * EV:(r + 1) * EV]
        wTn = np.ascontiguousarray(
            np.concatenate([wq_r.T, wk_r.T, wv_r.T], axis=1)).astype(bf)
        in_maps.append({
            "xT": xT, "wT": wTn, "woT": woT,
            "cosT": cosT, "sinPM": sinPM, "mask2": mask2, "ident": identm,
        })
    return in_maps


def kernel(x, freqs_cis, wq, wk, wv, wo):
    if "nc" not in _CACHE:
        _CACHE["nc"] = _build_nc()
    nc = _CACHE["nc"]

    in_maps = _host_prep(x, freqs_cis, wq, wk, wv, wo)
    trace = bool(int(os.environ.get("KPROF", "0")))
    res = run_bass_kernel_spmd(nc, in_maps, core_ids=list(range(NCORES)),
                               trace=trace)
    if trace:
        _CACHE["last_results"] = res

    full = np.empty((BS, D), np.float32)
    for r in range(NCORES):
        o = res.results[r]["out"]
        full[r * RSH:(r + 1) * RSH] = o[0:RSH]
        full[S + r * RSH:S + (r + 1) * RSH] = o[RSH:2 * RSH]
    return full.reshape(B, S, D)


if __name__ == "__main__":
    rng = np.random.default_rng(0)
    ins = {
        "x": rng.standard_normal((B, S, D), np.float32),
        "freqs_cis": rng.standard_normal((S, HD // 2, 2), np.float32),
        "wq": (rng.standard_normal((H * HD, D)) * 0.02).astype(np.float32),
        "wk": (rng.standard_normal((KV * HD, D)) * 0.02).astype(np.float32),
        "wv": (rng.standard_normal((KV * HD, D)) * 0.02).astype(np.float32),
        "wo": (rng.standard_normal((D, H * HD)) * 0.02).astype(np.float32),
    }
    out = kernel(**ins)
    print("kernel ran, out shape", out.shape, "finite:", np.isfinite(out).all())

